# revision 13
# baseline (speedup 1.0000x reference)
"""Trainium2 Bass kernel for nn_AspectLinkModel (BERT-ish dual encoder + pairwise cosine sum).

Strategy: pure data-parallel over batch (2 batches/core x 8 cores), both encoder
calls fused into one 768-token stream per core. Feature-major activations,
bf16 matmul operands (converted on host) with f32 accumulation. Output [16]
assembled on host from per-core per-sequence normalized sums (sum-then-dot).
"""
import sys
import numpy as np
import ml_dtypes

for _p in ('/opt/trn_rl_repo', '/root/.axon_site/_ro/trn_rl_repo'):
    if _p not in sys.path:
        sys.path.insert(0, _p)

import concourse.bass as bass  # noqa: E402
import concourse.tile as tile  # noqa: E402
from concourse import bacc, mybir  # noqa: E402
from concourse.bass_utils import run_bass_kernel_spmd  # noqa: E402

F32 = mybir.dt.float32
BF16 = mybir.dt.bfloat16
AF = mybir.ActivationFunctionType
AX = mybir.AxisListType

L, D, H, DH, FF, EMB = 4, 768, 12, 64, 3072, 300
KC = D // 128          # 6 feature chunks
FFC = FF // 128        # 24 ffn chunks
T = 768                # tokens per core: [ctx0(256) | ctx1(256) | asp0(128) | asp1(128)]
NTS = [(0, 384), (384, 384)]
CORES = list(range(8))

_NC_CACHE = {}


def _build_nc(n_layers=L, taps=(), with_bias=False):
    nc = bacc.Bacc("TRN2", target_bir_lowering=False, debug=False)

    dd = {}
    dd['xT_d'] = nc.dram_tensor("xT", [384, T], BF16, kind="ExternalInput")
    dd['fc1_d'] = nc.dram_tensor("fc1p", [384, D], BF16, kind="ExternalInput")
    dd['posT_d'] = nc.dram_tensor("posT", [D, T], BF16, kind="ExternalInput")
    dd['embg_d'] = nc.dram_tensor("emb_g", [D], F32, kind="ExternalInput")
    dd['embb_d'] = nc.dram_tensor("emb_b", [D], F32, kind="ExternalInput")
    dd['Wq_d'] = nc.dram_tensor("Wq", [L, D, D], BF16, kind="ExternalInput")
    dd['Wk_d'] = nc.dram_tensor("Wk", [L, D, D], BF16, kind="ExternalInput")
    dd['Wv_d'] = nc.dram_tensor("Wv", [L, D, D], BF16, kind="ExternalInput")
    dd['Wo_d'] = nc.dram_tensor("Wo", [L, D, D], BF16, kind="ExternalInput")
    dd['bq_d'] = nc.dram_tensor("bq", [L, D], F32, kind="ExternalInput")
    dd['bk_d'] = nc.dram_tensor("bk8", [L, D], F32, kind="ExternalInput")
    dd['bv_d'] = nc.dram_tensor("bv", [L, D], F32, kind="ExternalInput")
    dd['bo_d'] = nc.dram_tensor("bo", [L, D], F32, kind="ExternalInput")
    dd['l1g_d'] = nc.dram_tensor("ln1_g", [L, D], F32, kind="ExternalInput")
    dd['l1b_d'] = nc.dram_tensor("ln1_b", [L, D], F32, kind="ExternalInput")
    dd['l2g_d'] = nc.dram_tensor("ln2_g", [L, D], F32, kind="ExternalInput")
    dd['l2b_d'] = nc.dram_tensor("ln2_b", [L, D], F32, kind="ExternalInput")
    dd['W1r_d'] = nc.dram_tensor("W1r", [L, FFC, 128, D], BF16, kind="ExternalInput")
    dd['W2_d'] = nc.dram_tensor("W2", [L, FF, D], BF16, kind="ExternalInput")
    dd['bf1_d'] = nc.dram_tensor("bf1", [L, FF], F32, kind="ExternalInput")
    dd['bf2_d'] = nc.dram_tensor("bf2", [L, D], F32, kind="ExternalInput")
    dd['out_d'] = nc.dram_tensor("out", [128, 96], F32, kind="ExternalOutput")
    tap_d = {}
    for tname in taps:
        tap_d[tname] = nc.dram_tensor("tap_" + tname, [128, 6 * T], BF16,
                                      kind="ExternalOutput")

    with tile.TileContext(nc) as tc:
        _body(nc, tc, dd, n_layers, taps, tap_d, with_bias)
    nc.compile()
    return nc


def _body(nc, tc, dd, n_layers, taps, tap_d, with_bias):
    import contextlib
    ctx = contextlib.ExitStack()
    with ctx:
        sb = ctx.enter_context(tc.tile_pool(name="sb", bufs=1))
        ps = ctx.enter_context(tc.tile_pool(name="ps", bufs=1, space="PSUM"))

        # ---------------- constants ----------------
        ones_bf = sb.tile([128, 128], BF16, name="ones_bf", tag="const", bufs=4)
        nc.vector.memset(ones_bf[:], 1.0)
        eps12 = sb.tile([128, 1], F32, name="eps12", tag="const_e", bufs=2)
        nc.vector.memset(eps12[:], 1e-12)
        eps16 = sb.tile([128, 1], F32, name="eps16", tag="const_e2", bufs=2)
        nc.vector.memset(eps16[:], 1e-16)

        def col_tile(name, dram_row, n):
            t = sb.tile([128, n], F32, name=name, tag="cols", bufs=44)
            nc.sync.dma_start(t[:], dram_row.rearrange("(c p) -> p c", p=128))
            return t

        # 4 persistent big bf16 tiles [128, 6*768]; roles rotate by phase:
        # bigs[0]: posT -> qT -> f1[0..5];  bigs[1]: kT -> f1[6..11]
        # bigs[2]: vtok -> f1[12..17];      bigs[3]: oT -> f1[18..23]
        bigs = [sb.tile([128, KC * T], BF16, name=f"bigp{i}", tag=f"pers{i}", bufs=1)
                for i in range(4)]

        def hpool(name):
            return sb.tile([128, KC * T], BF16, name=name, tag="hmid", bufs=2)

        def saved_tile(name):
            return sb.tile([128, KC * T], BF16, name=name, tag="saved", bufs=4)

        def s_tile(name):
            return sb.tile([128, KC * T], BF16, name=name, tag="spre", bufs=2)

        def wstr(name):
            return sb.tile([128, D], BF16, name=name, tag="wstr", bufs=10)

        def bank(name, w=384):
            return ps.tile([128, w], F32, name=name, tag="bank", bufs=8)

        def stat(name):
            return sb.tile([128, 384], F32, name=name, tag="stat", bufs=8)

        def statw(name):
            return sb.tile([128, D], F32, name=name, tag="statw", bufs=3)

        def stbf(name):
            return sb.tile([128, 384], BF16, name=name, tag="stbf", bufs=4)

        # ---------------- LayerNorm (token-half pipelined) ----------------
        def layer_norm(pref, s, g_col, gc, b_col, bc, out):
            """s: [128,4608] bf16 pre-LN sums; out: [128,4608] bf16.
            Processed independently per token half so consumers of half 0
            can start while half 1 is still in flight."""
            for ih, (o, w) in enumerate(NTS):
                S1 = bank(f"{pref}_S1_{ih}")
                S2 = bank(f"{pref}_S2_{ih}")
                for c in range(KC):
                    sq = stbf(f"{pref}_sq{ih}_{c}")
                    nc.gpsimd.tensor_mul(sq[:], s[:, c * T + o:c * T + o + w],
                                         s[:, c * T + o:c * T + o + w])
                    nc.tensor.matmul(S1[:], ones_bf[:], s[:, c * T + o:c * T + o + w],
                                     start=(c == 0), stop=(c == KC - 1))
                    nc.tensor.matmul(S2[:], ones_bf[:], sq[:],
                                     start=(c == 0), stop=(c == KC - 1))
                mean = stat(f"{pref}_mean{ih}")
                m2 = stat(f"{pref}_m2{ih}")
                nc.scalar.mul(mean[:], S1[:], 1.0 / D)
                nc.scalar.mul(m2[:], S2[:], 1.0 / D)
                var = stat(f"{pref}_var{ih}")
                nc.vector.tensor_mul(var[:], mean[:], mean[:])
                nc.vector.tensor_sub(var[:], m2[:], var[:])
                # rstd = exp(-0.5 * ln(var + eps))  (ACT, keeps DVE free)
                lnv = stat(f"{pref}_lnv{ih}")
                nc.scalar.activation(lnv[:], var[:], AF.Ln, bias=eps12[:])
                rstd = stat(f"{pref}_rstd{ih}")
                nc.scalar.activation(rstd[:], lnv[:], AF.Exp, scale=-0.5)
                for c in range(KC):
                    eng = nc.gpsimd if c >= 4 else nc.vector
                    t1 = stat(f"{pref}_t1_{ih}_{c}")
                    eng.tensor_sub(t1[:], s[:, c * T + o:c * T + o + w], mean[:])
                    eng.tensor_mul(t1[:], t1[:], rstd[:])
                    nc.scalar.activation(out[:, c * T + o:c * T + o + w], t1[:],
                                         AF.Identity, bias=b_col[:, bc:bc + 1],
                                         scale=g_col[:, gc:gc + 1])

        # ---------------- embed: fc1 + pos + LN ----------------
        xT_bf = [wstr(f"xT{c}") for c in range(3)]
        fc1_bf = [wstr(f"fc1w{c}") for c in range(3)]
        for c in range(3):
            nc.sync.dma_start(xT_bf[c][:], dd['xT_d'][c * 128:(c + 1) * 128, :])
            nc.sync.dma_start(fc1_bf[c][:], dd['fc1_d'][c * 128:(c + 1) * 128, :])
        posT = bigs[0]
        for c in range(KC):
            nc.sync.dma_start(posT[:, c * T:(c + 1) * T],
                              dd['posT_d'][c * 128:(c + 1) * 128, :])
        embg = col_tile("embg", dd['embg_d'][:], KC)
        embb = col_tile("embb", dd['embb_d'][:], KC)

        s0 = s_tile("s_emb")
        for i, (o, w) in enumerate(NTS):
            for mc in range(KC):
                p = bank(f"emb_{mc}_{i}")
                for kc in range(3):
                    nc.tensor.matmul(p[:], fc1_bf[kc][:, mc * 128:(mc + 1) * 128],
                                     xT_bf[kc][:, o:o + w],
                                     start=(kc == 0), stop=(kc == 2))
                nc.vector.tensor_add(s0[:, mc * T + o:mc * T + o + w], p[:],
                                     posT[:, mc * T + o:mc * T + o + w])
        h = hpool("h0")
        layer_norm("ln_emb", s0, embg, 0, embb, 0, h)
        if 'h0' in taps:
            nc.sync.dma_start(tap_d['h0'][:], h[:])

        saved = []
        N2 = [bank(f"n2_{i}") for i in range(2)]

        # ---------------- transformer layers ----------------
        for l in range(n_layers):
            bq = col_tile(f"bq{l}", dd['bq_d'][l], KC)
            bk = col_tile(f"bk{l}", dd['bk_d'][l], KC)
            l1g = col_tile(f"l1g{l}", dd['l1g_d'][l], KC)
            l1b = col_tile(f"l1b{l}", dd['l1b_d'][l], KC)
            l2g = col_tile(f"l2g{l}", dd['l2g_d'][l], KC)
            l2b = col_tile(f"l2b{l}", dd['l2b_d'][l], KC)
            bf1c = col_tile(f"bf1{l}", dd['bf1_d'][l], FFC)
            if with_bias:
                bvc = col_tile(f"bvc{l}", dd['bv_d'][l], KC)
                boc = col_tile(f"boc{l}", dd['bo_d'][l], KC)
                bf2c = col_tile(f"bf2c{l}", dd['bf2_d'][l], KC)

            # ---- Q^T, K^T (feature-major, K pre-scaled 1/8) ----
            qT = bigs[0]
            kT = bigs[1]
            for dst, W_d, bias_col, scl, wn in ((qT, dd['Wq_d'], bq, 1.0, 'q'),
                                                (kT, dd['Wk_d'], bk, 0.125, 'k')):
                w_bf = [wstr(f"w{wn}{l}_{c}") for c in range(KC)]
                for c in range(KC):
                    nc.sync.dma_start(w_bf[c][:], W_d[l, c * 128:(c + 1) * 128, :])
                for i, (o, w) in enumerate(NTS):
                    for mc in range(KC):
                        p = bank(f"p{wn}{l}_{mc}_{i}")
                        for kc in range(KC):
                            nc.tensor.matmul(p[:], w_bf[kc][:, mc * 128:(mc + 1) * 128],
                                             h[:, kc * T + o:kc * T + o + w],
                                             start=(kc == 0), stop=(kc == KC - 1))
                        nc.scalar.activation(dst[:, mc * T + o:mc * T + o + w], p[:],
                                             AF.Identity, bias=bias_col[:, mc:mc + 1],
                                             scale=scl)

            # ---- V (token-major: [token_chunk rows, feature cols]) ----
            vtok = bigs[2]
            wv_bf = [wstr(f"wv{l}_{c}") for c in range(KC)]
            for c in range(KC):
                nc.sync.dma_start(wv_bf[c][:], dd['Wv_d'][l, c * 128:(c + 1) * 128, :])
            for tch in range(KC):
                for i, (o, w) in enumerate(NTS):
                    p = bank(f"pv{l}_{tch}_{i}")
                    for kc in range(KC):
                        nc.tensor.matmul(p[:], h[:, kc * T + tch * 128:kc * T + (tch + 1) * 128],
                                         wv_bf[kc][:, o:o + w],
                                         start=(kc == 0), stop=(kc == KC - 1))
                    nc.vector.tensor_copy(vtok[:, tch * T + o:tch * T + o + w], p[:])

            # ---- attention (heads paired per feature chunk) ----
            oT = bigs[3]
            for si, (qo, kts) in enumerate(((0, (0, 1)), (256, (2, 3)))):
                for c in range(KC):
                    av = bank(f"av{l}_{si}_{c}", 256)
                    cs = bank(f"cs{l}_{si}_{c}", 256)
                    eTs = []
                    for ki, kt in enumerate(kts):
                        eT = sb.tile([128, 512], BF16, name=f"eT{l}_{si}_{c}_{ki}",
                                     tag="eT", bufs=6)
                        for j in range(2):
                            sp = bank(f"sp{l}_{si}_{c}_{ki}_{j}", 256)
                            nc.tensor.matmul(
                                sp[:],
                                kT[j * 64:j * 64 + 64, c * T + kt * 128:c * T + (kt + 1) * 128],
                                qT[j * 64:j * 64 + 64, c * T + qo:c * T + qo + 256],
                                start=True, stop=True)
                            nc.scalar.activation(eT[:, j * 256:(j + 1) * 256], sp[:], AF.Exp)
                        eTs.append(eT)
                    for j in range(2):
                        hh = 2 * c + j
                        r0 = j * 64
                        for ki in range(2):
                            nc.tensor.matmul(av[r0:r0 + 64, :],
                                             vtok[:, kts[ki] * T + hh * 64:kts[ki] * T + hh * 64 + 64],
                                             eTs[ki][:, j * 256:(j + 1) * 256],
                                             start=(ki == 0), stop=(ki == 1),
                                             tile_position=(0, r0))
                        for ki in range(2):
                            nc.tensor.matmul(cs[r0:r0 + 64, :], ones_bf[:, 0:64],
                                             eTs[ki][:, j * 256:(j + 1) * 256],
                                             start=(ki == 0), stop=(ki == 1),
                                             tile_position=(0, r0))
                    # inv = exp(-ln(cs)) on ACT (frees DVE); multiply on DVE (PSUM)
                    lnc = sb.tile([128, 256], F32, name=f"lnc{l}_{si}_{c}",
                                  tag="inv", bufs=6)
                    nc.scalar.activation(lnc[:], cs[:], AF.Ln)
                    inv = sb.tile([128, 256], F32, name=f"inv{l}_{si}_{c}",
                                  tag="inv", bufs=6)
                    nc.scalar.activation(inv[:], lnc[:], AF.Exp, scale=-1.0)
                    nc.vector.tensor_mul(oT[:, c * T + qo:c * T + qo + 256], av[:], inv[:])
            # asp joint pass: q = [512:768] covers both asp seqs; keys per chunk
            for kt in (4, 5):
                qw0 = 512 + (kt - 4) * 128
                col0 = (kt - 4) * 128
                for c in range(KC):
                    eT = sb.tile([128, 512], BF16, name=f"eTa{l}_{kt}_{c}",
                                 tag="eT", bufs=6)
                    for j in range(2):
                        sp = bank(f"spa{l}_{kt}_{c}_{j}", 256)
                        nc.tensor.matmul(
                            sp[:],
                            kT[j * 64:j * 64 + 64, c * T + kt * 128:c * T + (kt + 1) * 128],
                            qT[j * 64:j * 64 + 64, c * T + 512:c * T + 768],
                            start=True, stop=True)
                        nc.scalar.activation(eT[:, j * 256:(j + 1) * 256], sp[:], AF.Exp)
                    av = bank(f"ava{l}_{kt}_{c}", 128)
                    cs = bank(f"csa{l}_{kt}_{c}", 128)
                    for j in range(2):
                        hh = 2 * c + j
                        r0 = j * 64
                        nc.tensor.matmul(av[r0:r0 + 64, :],
                                         vtok[:, kt * T + hh * 64:kt * T + hh * 64 + 64],
                                         eT[:, j * 256 + col0:j * 256 + col0 + 128],
                                         start=True, stop=True, tile_position=(0, r0))
                        nc.tensor.matmul(cs[r0:r0 + 64, :], ones_bf[:, 0:64],
                                         eT[:, j * 256 + col0:j * 256 + col0 + 128],
                                         start=True, stop=True, tile_position=(0, r0))
                    lnc = sb.tile([128, 256], F32, name=f"lnca{l}_{kt}_{c}",
                                  tag="inv", bufs=6)
                    nc.scalar.activation(lnc[:, 0:128], cs[:], AF.Ln)
                    inv = sb.tile([128, 256], F32, name=f"inva{l}_{kt}_{c}",
                                  tag="inv", bufs=6)
                    nc.scalar.activation(inv[:, 0:128], lnc[:, 0:128], AF.Exp, scale=-1.0)
                    nc.vector.tensor_mul(oT[:, c * T + qw0:c * T + qw0 + 128],
                                         av[:], inv[:, 0:128])

            if with_bias:
                for c in range(KC):
                    nc.scalar.activation(oT[:, c * T:(c + 1) * T],
                                         oT[:, c * T:(c + 1) * T], AF.Identity,
                                         bias=bvc[:, c:c + 1])

            # ---- O projection + residual -> s1, LN1 -> hmid ----
            wo_bf = [wstr(f"wo{l}_{c}") for c in range(KC)]
            for c in range(KC):
                nc.sync.dma_start(wo_bf[c][:], dd['Wo_d'][l, c * 128:(c + 1) * 128, :])
            s1 = s_tile(f"s1_{l}")
            for i, (o, w) in enumerate(NTS):
                for mc in range(KC):
                    p = bank(f"po{l}_{mc}_{i}")
                    for kc in range(KC):
                        nc.tensor.matmul(p[:], wo_bf[kc][:, mc * 128:(mc + 1) * 128],
                                         oT[:, kc * T + o:kc * T + o + w],
                                         start=(kc == 0), stop=(kc == KC - 1))
                    nc.vector.tensor_add(s1[:, mc * T + o:mc * T + o + w], p[:],
                                         h[:, mc * T + o:mc * T + o + w])
                    if with_bias:
                        nc.scalar.activation(s1[:, mc * T + o:mc * T + o + w],
                                             s1[:, mc * T + o:mc * T + o + w],
                                             AF.Identity, bias=boc[:, mc:mc + 1])
            hmid = hpool(f"hmid{l}")
            layer_norm(f"ln1_{l}", s1, l1g, 0, l1b, 0, hmid)

            # ---- FFN ----
            w2all = sb.tile([128, FFC * T], BF16, name=f"w2all{l}", tag="w2all", bufs=1)
            for fc in range(FFC):
                nc.sync.dma_start(w2all[:, fc * T:(fc + 1) * T],
                                  dd['W2_d'][l, fc * 128:(fc + 1) * 128, :])
            f1 = bigs
            for fc in range(FFC):
                w1c = wstr(f"w1_{l}_{fc}")
                nc.sync.dma_start(w1c[:], dd['W1r_d'][l, fc])
                ftile, foff = f1[fc // 6], (fc % 6) * T
                for i, (o, w) in enumerate(NTS):
                    p = bank(f"pf1{l}_{fc}_{i}")
                    for kc in range(KC):
                        nc.tensor.matmul(p[:], w1c[:, kc * 128:(kc + 1) * 128],
                                         hmid[:, kc * T + o:kc * T + o + w],
                                         start=(kc == 0), stop=(kc == KC - 1))
                    nc.scalar.activation(ftile[:, foff + o:foff + o + w], p[:],
                                         AF.Gelu_apprx_tanh, bias=bf1c[:, fc:fc + 1])
            s2 = s_tile(f"s2_{l}")
            for i, (o, w) in enumerate(NTS):
                for mc in range(KC):
                    p = bank(f"pf2{l}_{mc}_{i}")
                    for fc in range(FFC):
                        ftile, foff = f1[fc // 6], (fc % 6) * T
                        nc.tensor.matmul(
                            p[:], w2all[:, fc * T + mc * 128:fc * T + (mc + 1) * 128],
                            ftile[:, foff + o:foff + o + w],
                            start=(fc == 0), stop=(fc == FFC - 1))
                    nc.vector.tensor_add(s2[:, mc * T + o:mc * T + o + w], p[:],
                                         hmid[:, mc * T + o:mc * T + o + w])
                    if with_bias:
                        nc.scalar.activation(s2[:, mc * T + o:mc * T + o + w],
                                             s2[:, mc * T + o:mc * T + o + w],
                                             AF.Identity, bias=bf2c[:, mc:mc + 1])
            hout = saved_tile(f"hL{l}")
            layer_norm(f"ln2_{l}", s2, l2g, 0, l2b, 0, hout)
            # accumulate squared-norm contribution for the final cosine stage
            for c in range(KC):
                for i, (o, w) in enumerate(NTS):
                    sq = stbf(f"fin_sq{l}_{c}_{i}")
                    nc.gpsimd.tensor_mul(sq[:], hout[:, c * T + o:c * T + o + w],
                                         hout[:, c * T + o:c * T + o + w])
                    nc.tensor.matmul(N2[i][:], ones_bf[:], sq[:],
                                     start=(l == 0 and c == 0),
                                     stop=(l == n_layers - 1 and c == KC - 1))
            saved.append(hout)
            h = hout
            if f"h{l + 1}" in taps:
                nc.sync.dma_start(tap_d[f"h{l + 1}"][:], hout[:])

        # ---------------- final: weighted per-seq sums ----------------
        invn = statw("invn")
        for i, (o, w) in enumerate(NTS):
            sd = stat(f"fin_sd{i}")
            nc.scalar.activation(sd[:], N2[i][:], AF.Sqrt, bias=eps16[:])
            nc.vector.reciprocal(invn[:, o:o + w], sd[:])
        outb = sb.tile([128, 96], F32, name="outb", tag="outb", bufs=1)
        nc.vector.memset(outb[:], 0.0)
        for li, hs in enumerate(saved):
            for c in range(KC):
                wt = statw(f"fin_w{li}_{c}")
                eng = nc.vector if (c % 2 == 0) else nc.gpsimd
                eng.tensor_mul(wt[:], hs[:, c * T:(c + 1) * T], invn[:])
                base = (li * KC + c) * 4
                nc.vector.reduce_sum(outb[:, base:base + 2],
                                     wt[:, 0:512].rearrange("p (s q) -> p s q", s=2),
                                     axis=AX.X)
                nc.vector.reduce_sum(outb[:, base + 2:base + 4],
                                     wt[:, 512:768].rearrange("p (s q) -> p s q", s=2),
                                     axis=AX.X)
        nc.sync.dma_start(dd['out_d'][:], outb[:])


def _prep_in_maps(inputs):
    f = lambda k: np.ascontiguousarray(np.asarray(inputs[k], np.float32))
    bf = lambda a: np.ascontiguousarray(np.asarray(a, ml_dtypes.bfloat16))
    ctx_e, asp_e = f('context_inputs_embeds'), f('aspect_inputs_embeds')
    fc1_w, fc1_b = f('fc1_w'), f('fc1_b')
    pos = f('pos_emb')

    fc1p = np.zeros((384, D), np.float32)
    fc1p[:EMB] = fc1_w
    posT = np.concatenate([pos[:256].T, pos[:256].T, pos[:128].T, pos[:128].T],
                          axis=1) + fc1_b[:, None]
    W1 = f('W1')
    # [L, FFC, 128, 768]: W1r[l, fc, p, kc*128+c2] = W1[l, kc*128+p, fc*128+c2]
    W1r = W1.reshape(L, KC, 128, FFC, 128).transpose(0, 3, 2, 1, 4).reshape(L, FFC, 128, D)
    common = {
        'fc1p': bf(fc1p), 'posT': bf(posT),
        'emb_g': f('emb_g'), 'emb_b': f('emb_b'),
        'Wq': bf(f('Wq')), 'Wk': bf(f('Wk')), 'Wv': bf(f('Wv')), 'Wo': bf(f('Wo')),
        'bq': f('bq'), 'bk8': f('bk') * 0.125, 'bv': f('bv'), 'bo': f('bo'),
        'ln1_g': f('ln1_g'), 'ln1_b': f('ln1_b'),
        'ln2_g': f('ln2_g'), 'ln2_b': f('ln2_b'),
        'W1r': bf(W1r), 'W2': bf(f('W2')), 'bf1': f('bf1'), 'bf2': f('bf2'),
    }
    in_maps = []
    for i in range(8):
        xT = np.zeros((384, T), np.float32)
        xT[:EMB, 0:256] = ctx_e[2 * i].T
        xT[:EMB, 256:512] = ctx_e[2 * i + 1].T
        xT[:EMB, 512:640] = asp_e[2 * i].T
        xT[:EMB, 640:768] = asp_e[2 * i + 1].T
        in_maps.append({**common, 'xT': bf(xT)})
    return in_maps


def _postprocess(results):
    out = np.zeros(16, np.float32)
    for i, r in enumerate(results):
        A = r['out'].reshape(128, 24, 4)
        for j in range(2):
            out[2 * i + j] = float(np.sum(
                A[:, :, 2 + j].astype(np.float64) * A[:, :, j].astype(np.float64)))
    return out


def get_nc(n_layers=L, taps=(), with_bias=False):
    key = (n_layers, tuple(taps), with_bias)
    if key not in _NC_CACHE:
        _NC_CACHE[key] = _build_nc(n_layers, taps, with_bias)
    return _NC_CACHE[key]


def kernel(**inputs):
    wb = any(float(np.abs(np.asarray(inputs[k])).max()) > 0
             for k in ('bv', 'bo', 'bf2'))
    nc = get_nc(with_bias=wb)
    in_maps = _prep_in_maps(inputs)
    res = run_bass_kernel_spmd(nc, in_maps, CORES)
    return _postprocess(res.results)


if __name__ == "__main__":
    d = np.load('/root/problem/inputs_cache.npz')
    out = kernel(**{k: d[k] for k in d.files})
    ref = np.load('/root/problem/ref_out.npy')
    rel = np.abs(out - ref) / np.abs(ref)
    print("out:", out)
    print("rel err:", rel.max())


# revision 14
# speedup vs baseline: 1.2599x; 1.2599x over previous
"""Trainium2 Bass kernel for nn_AspectLinkModel (BERT-ish dual encoder + pairwise cosine sum).

Strategy: pure data-parallel over batch (2 batches/core x 8 cores), both encoder
calls fused into one 768-token stream per core. Feature-major activations,
bf16 matmul operands (converted on host) with f32 accumulation. Output [16]
assembled on host from per-core per-sequence normalized sums (sum-then-dot).
"""
import sys
import numpy as np
import ml_dtypes

for _p in ('/opt/trn_rl_repo', '/root/.axon_site/_ro/trn_rl_repo'):
    if _p not in sys.path:
        sys.path.insert(0, _p)

import concourse.bass as bass  # noqa: E402
import concourse.tile as tile  # noqa: E402
from concourse import bacc, mybir  # noqa: E402
from concourse.bass_utils import run_bass_kernel_spmd  # noqa: E402

# Steer the ACT table-set chooser so interleaved Exp/Ln activations resolve to
# the combined natural_log_exp_and_others set instead of thrashing between
# exp_and_others and natural_log (~2.7us per reload). Set order (and therefore
# act_func_set_id indices) is preserved; we only hide exp/ln from the
# single-function sets.
import concourse.hw_specs as _hw_specs  # noqa: E402
import functools  # noqa: E402

_orig_get_tables = _hw_specs.get_activation_tables


@functools.cache
def _patched_get_tables(arch):
    src = _orig_get_tables(arch)
    out = {}
    for name, funcs in src.items():
        fs = set(funcs)
        if name == 'exp_and_others':
            fs.discard(mybir.ActivationFunctionType.Exp)
        if name == 'natural_log':
            fs.discard(mybir.ActivationFunctionType.Ln)
        out[name] = fs
    return out


_hw_specs.get_activation_tables = _patched_get_tables
if getattr(bacc, 'get_activation_tables', None) is _orig_get_tables:
    bacc.get_activation_tables = _patched_get_tables

F32 = mybir.dt.float32
BF16 = mybir.dt.bfloat16
AF = mybir.ActivationFunctionType
AX = mybir.AxisListType

L, D, H, DH, FF, EMB = 4, 768, 12, 64, 3072, 300
KC = D // 128          # 6 feature chunks
FFC = FF // 128        # 24 ffn chunks
T = 768                # tokens per core: [ctx0(256) | ctx1(256) | asp0(128) | asp1(128)]
NTS = [(0, 384), (384, 384)]
CORES = list(range(8))

_NC_CACHE = {}


def _build_nc(n_layers=L, taps=(), with_bias=False):
    nc = bacc.Bacc("TRN2", target_bir_lowering=False, debug=False)

    dd = {}
    dd['xT_d'] = nc.dram_tensor("xT", [384, T], BF16, kind="ExternalInput")
    dd['fc1_d'] = nc.dram_tensor("fc1p", [384, D], BF16, kind="ExternalInput")
    dd['posT_d'] = nc.dram_tensor("posT", [D, T], BF16, kind="ExternalInput")
    dd['embg_d'] = nc.dram_tensor("emb_g", [D], F32, kind="ExternalInput")
    dd['embb_d'] = nc.dram_tensor("emb_b", [D], F32, kind="ExternalInput")
    dd['Wq_d'] = nc.dram_tensor("Wq", [L, D, D], BF16, kind="ExternalInput")
    dd['Wk_d'] = nc.dram_tensor("Wk", [L, D, D], BF16, kind="ExternalInput")
    dd['Wv_d'] = nc.dram_tensor("Wv", [L, D, D], BF16, kind="ExternalInput")
    dd['Wo_d'] = nc.dram_tensor("Wo", [L, D, D], BF16, kind="ExternalInput")
    dd['bq_d'] = nc.dram_tensor("bq", [L, D], F32, kind="ExternalInput")
    dd['bk_d'] = nc.dram_tensor("bk8", [L, D], F32, kind="ExternalInput")
    dd['bv_d'] = nc.dram_tensor("bv", [L, D], F32, kind="ExternalInput")
    dd['bo_d'] = nc.dram_tensor("bo", [L, D], F32, kind="ExternalInput")
    dd['l1g_d'] = nc.dram_tensor("ln1_g", [L, D], F32, kind="ExternalInput")
    dd['l1b_d'] = nc.dram_tensor("ln1_b", [L, D], F32, kind="ExternalInput")
    dd['l2g_d'] = nc.dram_tensor("ln2_g", [L, D], F32, kind="ExternalInput")
    dd['l2b_d'] = nc.dram_tensor("ln2_b", [L, D], F32, kind="ExternalInput")
    dd['W1r_d'] = nc.dram_tensor("W1r", [L, FFC, 128, D], BF16, kind="ExternalInput")
    dd['W2_d'] = nc.dram_tensor("W2", [L, FF, D], BF16, kind="ExternalInput")
    dd['bf1_d'] = nc.dram_tensor("bf1", [L, FF], F32, kind="ExternalInput")
    dd['bf2_d'] = nc.dram_tensor("bf2", [L, D], F32, kind="ExternalInput")
    dd['out_d'] = nc.dram_tensor("out", [128, 96], F32, kind="ExternalOutput")
    tap_d = {}
    for tname in taps:
        tap_d[tname] = nc.dram_tensor("tap_" + tname, [128, 6 * T], BF16,
                                      kind="ExternalOutput")

    with tile.TileContext(nc) as tc:
        _body(nc, tc, dd, n_layers, taps, tap_d, with_bias)
    nc.compile()
    return nc


def _body(nc, tc, dd, n_layers, taps, tap_d, with_bias):
    import contextlib
    ctx = contextlib.ExitStack()
    with ctx:
        sb = ctx.enter_context(tc.tile_pool(name="sb", bufs=1))
        ps = ctx.enter_context(tc.tile_pool(name="ps", bufs=1, space="PSUM"))

        # ---------------- constants ----------------
        ones_bf = sb.tile([128, 128], BF16, name="ones_bf", tag="const", bufs=4)
        nc.vector.memset(ones_bf[:], 1.0)
        eps12 = sb.tile([128, 1], F32, name="eps12", tag="const_e", bufs=2)
        nc.vector.memset(eps12[:], 1e-12)
        eps16 = sb.tile([128, 1], F32, name="eps16", tag="const_e2", bufs=2)
        nc.vector.memset(eps16[:], 1e-16)

        def col_tile(name, dram_row, n):
            t = sb.tile([128, n], F32, name=name, tag="cols", bufs=44)
            nc.sync.dma_start(t[:], dram_row.rearrange("(c p) -> p c", p=128))
            return t

        # 4 persistent big bf16 tiles [128, 6*768]; roles rotate by phase:
        # bigs[0]: posT -> qT -> f1[0..5];  bigs[1]: kT -> f1[6..11]
        # bigs[2]: vtok -> f1[12..17];      bigs[3]: oT -> f1[18..23]
        bigs = [sb.tile([128, KC * T], BF16, name=f"bigp{i}", tag=f"pers{i}", bufs=1)
                for i in range(4)]

        def hpool(name):
            return sb.tile([128, KC * T], BF16, name=name, tag="hmid", bufs=2)

        def saved_tile(name):
            return sb.tile([128, KC * T], BF16, name=name, tag="saved", bufs=4)

        def s_tile(name):
            return sb.tile([128, KC * T], BF16, name=name, tag="spre", bufs=2)

        def wstr(name):
            return sb.tile([128, D], BF16, name=name, tag="wstr", bufs=10)

        def bank(name, w=384):
            return ps.tile([128, w], F32, name=name, tag="bank", bufs=8)

        def stat(name):
            return sb.tile([128, 384], F32, name=name, tag="stat", bufs=8)

        def statw(name):
            return sb.tile([128, D], F32, name=name, tag="statw", bufs=3)

        def stbf(name):
            return sb.tile([128, 384], BF16, name=name, tag="stbf", bufs=4)

        # ---------------- LayerNorm (token-half pipelined) ----------------
        def layer_norm(pref, s, g_col, gc, b_col, bc, out):
            """s: [128,4608] bf16 pre-LN sums; out: [128,4608] bf16.
            Processed independently per token half so consumers of half 0
            can start while half 1 is still in flight."""
            for ih, (o, w) in enumerate(NTS):
                S1 = bank(f"{pref}_S1_{ih}")
                S2 = bank(f"{pref}_S2_{ih}")
                for c in range(KC):
                    sq = stbf(f"{pref}_sq{ih}_{c}")
                    nc.gpsimd.tensor_mul(sq[:], s[:, c * T + o:c * T + o + w],
                                         s[:, c * T + o:c * T + o + w])
                    nc.tensor.matmul(S1[:], ones_bf[:], s[:, c * T + o:c * T + o + w],
                                     start=(c == 0), stop=(c == KC - 1))
                    nc.tensor.matmul(S2[:], ones_bf[:], sq[:],
                                     start=(c == 0), stop=(c == KC - 1))
                mean = stat(f"{pref}_mean{ih}")
                m2 = stat(f"{pref}_m2{ih}")
                nc.scalar.mul(mean[:], S1[:], 1.0 / D)
                nc.scalar.mul(m2[:], S2[:], 1.0 / D)
                var = stat(f"{pref}_var{ih}")
                nc.vector.tensor_mul(var[:], mean[:], mean[:])
                nc.vector.tensor_sub(var[:], m2[:], var[:])
                # rstd = exp(-0.5 * ln(var + eps))  (ACT, keeps DVE free)
                lnv = stat(f"{pref}_lnv{ih}")
                nc.scalar.activation(lnv[:], var[:], AF.Ln, bias=eps12[:])
                rstd = stat(f"{pref}_rstd{ih}")
                nc.scalar.activation(rstd[:], lnv[:], AF.Exp, scale=-0.5)
                for c in range(KC):
                    eng = nc.gpsimd if c >= 4 else nc.vector
                    t1 = stat(f"{pref}_t1_{ih}_{c}")
                    eng.tensor_sub(t1[:], s[:, c * T + o:c * T + o + w], mean[:])
                    eng.tensor_mul(t1[:], t1[:], rstd[:])
                    nc.scalar.activation(out[:, c * T + o:c * T + o + w], t1[:],
                                         AF.Identity, bias=b_col[:, bc:bc + 1],
                                         scale=g_col[:, gc:gc + 1])

        # ---------------- embed: fc1 + pos + LN ----------------
        xT_bf = [wstr(f"xT{c}") for c in range(3)]
        fc1_bf = [wstr(f"fc1w{c}") for c in range(3)]
        for c in range(3):
            nc.sync.dma_start(xT_bf[c][:], dd['xT_d'][c * 128:(c + 1) * 128, :])
            nc.sync.dma_start(fc1_bf[c][:], dd['fc1_d'][c * 128:(c + 1) * 128, :])
        posT = bigs[0]
        for c in range(KC):
            nc.sync.dma_start(posT[:, c * T:(c + 1) * T],
                              dd['posT_d'][c * 128:(c + 1) * 128, :])
        embg = col_tile("embg", dd['embg_d'][:], KC)
        embb = col_tile("embb", dd['embb_d'][:], KC)

        s0 = s_tile("s_emb")
        for i, (o, w) in enumerate(NTS):
            for mc in range(KC):
                p = bank(f"emb_{mc}_{i}")
                for kc in range(3):
                    nc.tensor.matmul(p[:], fc1_bf[kc][:, mc * 128:(mc + 1) * 128],
                                     xT_bf[kc][:, o:o + w],
                                     start=(kc == 0), stop=(kc == 2))
                nc.vector.tensor_add(s0[:, mc * T + o:mc * T + o + w], p[:],
                                     posT[:, mc * T + o:mc * T + o + w])
        h = hpool("h0")
        layer_norm("ln_emb", s0, embg, 0, embb, 0, h)
        if 'h0' in taps:
            nc.sync.dma_start(tap_d['h0'][:], h[:])

        saved = []
        N2 = [bank(f"n2_{i}") for i in range(2)]

        # ---------------- transformer layers ----------------
        for l in range(n_layers):
            bq = col_tile(f"bq{l}", dd['bq_d'][l], KC)
            bk = col_tile(f"bk{l}", dd['bk_d'][l], KC)
            l1g = col_tile(f"l1g{l}", dd['l1g_d'][l], KC)
            l1b = col_tile(f"l1b{l}", dd['l1b_d'][l], KC)
            l2g = col_tile(f"l2g{l}", dd['l2g_d'][l], KC)
            l2b = col_tile(f"l2b{l}", dd['l2b_d'][l], KC)
            bf1c = col_tile(f"bf1{l}", dd['bf1_d'][l], FFC)
            if with_bias:
                bvc = col_tile(f"bvc{l}", dd['bv_d'][l], KC)
                boc = col_tile(f"boc{l}", dd['bo_d'][l], KC)
                bf2c = col_tile(f"bf2c{l}", dd['bf2_d'][l], KC)

            # ---- Q^T, K^T (feature-major, K pre-scaled 1/8) ----
            qT = bigs[0]
            kT = bigs[1]
            for dst, W_d, bias_col, scl, wn in ((qT, dd['Wq_d'], bq, 1.0, 'q'),
                                                (kT, dd['Wk_d'], bk, 0.125, 'k')):
                w_bf = [wstr(f"w{wn}{l}_{c}") for c in range(KC)]
                for c in range(KC):
                    nc.sync.dma_start(w_bf[c][:], W_d[l, c * 128:(c + 1) * 128, :])
                for i, (o, w) in enumerate(NTS):
                    for mc in range(KC):
                        p = bank(f"p{wn}{l}_{mc}_{i}")
                        for kc in range(KC):
                            nc.tensor.matmul(p[:], w_bf[kc][:, mc * 128:(mc + 1) * 128],
                                             h[:, kc * T + o:kc * T + o + w],
                                             start=(kc == 0), stop=(kc == KC - 1))
                        nc.scalar.activation(dst[:, mc * T + o:mc * T + o + w], p[:],
                                             AF.Identity, bias=bias_col[:, mc:mc + 1],
                                             scale=scl)

            # ---- V (token-major: [token_chunk rows, feature cols]) ----
            vtok = bigs[2]
            wv_bf = [wstr(f"wv{l}_{c}") for c in range(KC)]
            for c in range(KC):
                nc.sync.dma_start(wv_bf[c][:], dd['Wv_d'][l, c * 128:(c + 1) * 128, :])
            for tch in range(KC):
                for i, (o, w) in enumerate(NTS):
                    p = bank(f"pv{l}_{tch}_{i}")
                    for kc in range(KC):
                        nc.tensor.matmul(p[:], h[:, kc * T + tch * 128:kc * T + (tch + 1) * 128],
                                         wv_bf[kc][:, o:o + w],
                                         start=(kc == 0), stop=(kc == KC - 1))
                    nc.vector.tensor_copy(vtok[:, tch * T + o:tch * T + o + w], p[:])

            # ---- attention (heads paired per feature chunk) ----
            oT = bigs[3]
            for si, (qo, kts) in enumerate(((0, (0, 1)), (256, (2, 3)))):
                for c in range(KC):
                    av = bank(f"av{l}_{si}_{c}", 256)
                    cs = bank(f"cs{l}_{si}_{c}", 256)
                    eTs = []
                    for ki, kt in enumerate(kts):
                        eT = sb.tile([128, 512], BF16, name=f"eT{l}_{si}_{c}_{ki}",
                                     tag="eT", bufs=6)
                        for j in range(2):
                            sp = bank(f"sp{l}_{si}_{c}_{ki}_{j}", 256)
                            nc.tensor.matmul(
                                sp[:],
                                kT[j * 64:j * 64 + 64, c * T + kt * 128:c * T + (kt + 1) * 128],
                                qT[j * 64:j * 64 + 64, c * T + qo:c * T + qo + 256],
                                start=True, stop=True)
                            nc.scalar.activation(eT[:, j * 256:(j + 1) * 256], sp[:], AF.Exp)
                        eTs.append(eT)
                    for j in range(2):
                        hh = 2 * c + j
                        r0 = j * 64
                        for ki in range(2):
                            nc.tensor.matmul(av[r0:r0 + 64, :],
                                             vtok[:, kts[ki] * T + hh * 64:kts[ki] * T + hh * 64 + 64],
                                             eTs[ki][:, j * 256:(j + 1) * 256],
                                             start=(ki == 0), stop=(ki == 1),
                                             tile_position=(0, r0))
                        for ki in range(2):
                            nc.tensor.matmul(cs[r0:r0 + 64, :], ones_bf[:, 0:64],
                                             eTs[ki][:, j * 256:(j + 1) * 256],
                                             start=(ki == 0), stop=(ki == 1),
                                             tile_position=(0, r0))
                    # inv = exp(-ln(cs)) on ACT (frees DVE); multiply on DVE (PSUM)
                    lnc = sb.tile([128, 256], F32, name=f"lnc{l}_{si}_{c}",
                                  tag="inv", bufs=6)
                    nc.scalar.activation(lnc[:], cs[:], AF.Ln)
                    inv = sb.tile([128, 256], F32, name=f"inv{l}_{si}_{c}",
                                  tag="inv", bufs=6)
                    nc.scalar.activation(inv[:], lnc[:], AF.Exp, scale=-1.0)
                    nc.vector.tensor_mul(oT[:, c * T + qo:c * T + qo + 256], av[:], inv[:])
            # asp joint pass: q = [512:768] covers both asp seqs; keys per chunk
            for kt in (4, 5):
                qw0 = 512 + (kt - 4) * 128
                col0 = (kt - 4) * 128
                for c in range(KC):
                    eT = sb.tile([128, 512], BF16, name=f"eTa{l}_{kt}_{c}",
                                 tag="eT", bufs=6)
                    for j in range(2):
                        sp = bank(f"spa{l}_{kt}_{c}_{j}", 256)
                        nc.tensor.matmul(
                            sp[:],
                            kT[j * 64:j * 64 + 64, c * T + kt * 128:c * T + (kt + 1) * 128],
                            qT[j * 64:j * 64 + 64, c * T + 512:c * T + 768],
                            start=True, stop=True)
                        nc.scalar.activation(eT[:, j * 256:(j + 1) * 256], sp[:], AF.Exp)
                    av = bank(f"ava{l}_{kt}_{c}", 128)
                    cs = bank(f"csa{l}_{kt}_{c}", 128)
                    for j in range(2):
                        hh = 2 * c + j
                        r0 = j * 64
                        nc.tensor.matmul(av[r0:r0 + 64, :],
                                         vtok[:, kt * T + hh * 64:kt * T + hh * 64 + 64],
                                         eT[:, j * 256 + col0:j * 256 + col0 + 128],
                                         start=True, stop=True, tile_position=(0, r0))
                        nc.tensor.matmul(cs[r0:r0 + 64, :], ones_bf[:, 0:64],
                                         eT[:, j * 256 + col0:j * 256 + col0 + 128],
                                         start=True, stop=True, tile_position=(0, r0))
                    lnc = sb.tile([128, 256], F32, name=f"lnca{l}_{kt}_{c}",
                                  tag="inv", bufs=6)
                    nc.scalar.activation(lnc[:, 0:128], cs[:], AF.Ln)
                    inv = sb.tile([128, 256], F32, name=f"inva{l}_{kt}_{c}",
                                  tag="inv", bufs=6)
                    nc.scalar.activation(inv[:, 0:128], lnc[:, 0:128], AF.Exp, scale=-1.0)
                    nc.vector.tensor_mul(oT[:, c * T + qw0:c * T + qw0 + 128],
                                         av[:], inv[:, 0:128])

            if with_bias:
                for c in range(KC):
                    nc.scalar.activation(oT[:, c * T:(c + 1) * T],
                                         oT[:, c * T:(c + 1) * T], AF.Identity,
                                         bias=bvc[:, c:c + 1])

            # ---- O projection + residual -> s1, LN1 -> hmid ----
            wo_bf = [wstr(f"wo{l}_{c}") for c in range(KC)]
            for c in range(KC):
                nc.sync.dma_start(wo_bf[c][:], dd['Wo_d'][l, c * 128:(c + 1) * 128, :])
            s1 = s_tile(f"s1_{l}")
            for i, (o, w) in enumerate(NTS):
                for mc in range(KC):
                    p = bank(f"po{l}_{mc}_{i}")
                    for kc in range(KC):
                        nc.tensor.matmul(p[:], wo_bf[kc][:, mc * 128:(mc + 1) * 128],
                                         oT[:, kc * T + o:kc * T + o + w],
                                         start=(kc == 0), stop=(kc == KC - 1))
                    nc.vector.tensor_add(s1[:, mc * T + o:mc * T + o + w], p[:],
                                         h[:, mc * T + o:mc * T + o + w])
                    if with_bias:
                        nc.scalar.activation(s1[:, mc * T + o:mc * T + o + w],
                                             s1[:, mc * T + o:mc * T + o + w],
                                             AF.Identity, bias=boc[:, mc:mc + 1])
            hmid = hpool(f"hmid{l}")
            layer_norm(f"ln1_{l}", s1, l1g, 0, l1b, 0, hmid)

            # ---- FFN ----
            w2all = sb.tile([128, FFC * T], BF16, name=f"w2all{l}", tag="w2all", bufs=1)
            for fc in range(FFC):
                nc.sync.dma_start(w2all[:, fc * T:(fc + 1) * T],
                                  dd['W2_d'][l, fc * 128:(fc + 1) * 128, :])
            f1 = bigs
            for fc in range(FFC):
                w1c = wstr(f"w1_{l}_{fc}")
                nc.sync.dma_start(w1c[:], dd['W1r_d'][l, fc])
                ftile, foff = f1[fc // 6], (fc % 6) * T
                for i, (o, w) in enumerate(NTS):
                    p = bank(f"pf1{l}_{fc}_{i}")
                    for kc in range(KC):
                        nc.tensor.matmul(p[:], w1c[:, kc * 128:(kc + 1) * 128],
                                         hmid[:, kc * T + o:kc * T + o + w],
                                         start=(kc == 0), stop=(kc == KC - 1))
                    nc.scalar.activation(ftile[:, foff + o:foff + o + w], p[:],
                                         AF.Gelu_apprx_tanh, bias=bf1c[:, fc:fc + 1])
            s2 = s_tile(f"s2_{l}")
            for i, (o, w) in enumerate(NTS):
                for mc in range(KC):
                    p = bank(f"pf2{l}_{mc}_{i}")
                    for fc in range(FFC):
                        ftile, foff = f1[fc // 6], (fc % 6) * T
                        nc.tensor.matmul(
                            p[:], w2all[:, fc * T + mc * 128:fc * T + (mc + 1) * 128],
                            ftile[:, foff + o:foff + o + w],
                            start=(fc == 0), stop=(fc == FFC - 1))
                    nc.vector.tensor_add(s2[:, mc * T + o:mc * T + o + w], p[:],
                                         hmid[:, mc * T + o:mc * T + o + w])
                    if with_bias:
                        nc.scalar.activation(s2[:, mc * T + o:mc * T + o + w],
                                             s2[:, mc * T + o:mc * T + o + w],
                                             AF.Identity, bias=bf2c[:, mc:mc + 1])
            hout = saved_tile(f"hL{l}")
            layer_norm(f"ln2_{l}", s2, l2g, 0, l2b, 0, hout)
            # accumulate squared-norm contribution for the final cosine stage
            for c in range(KC):
                for i, (o, w) in enumerate(NTS):
                    sq = stbf(f"fin_sq{l}_{c}_{i}")
                    nc.gpsimd.tensor_mul(sq[:], hout[:, c * T + o:c * T + o + w],
                                         hout[:, c * T + o:c * T + o + w])
                    nc.tensor.matmul(N2[i][:], ones_bf[:], sq[:],
                                     start=(l == 0 and c == 0),
                                     stop=(l == n_layers - 1 and c == KC - 1))
            saved.append(hout)
            h = hout
            if f"h{l + 1}" in taps:
                nc.sync.dma_start(tap_d[f"h{l + 1}"][:], hout[:])

        # ---------------- final: weighted per-seq sums ----------------
        invn = statw("invn")
        for i, (o, w) in enumerate(NTS):
            sd = stat(f"fin_sd{i}")
            nc.scalar.activation(sd[:], N2[i][:], AF.Sqrt, bias=eps16[:])
            nc.vector.reciprocal(invn[:, o:o + w], sd[:])
        outb = sb.tile([128, 96], F32, name="outb", tag="outb", bufs=1)
        nc.vector.memset(outb[:], 0.0)
        for li, hs in enumerate(saved):
            for c in range(KC):
                wt = statw(f"fin_w{li}_{c}")
                eng = nc.vector if (c % 2 == 0) else nc.gpsimd
                eng.tensor_mul(wt[:], hs[:, c * T:(c + 1) * T], invn[:])
                base = (li * KC + c) * 4
                nc.vector.reduce_sum(outb[:, base:base + 2],
                                     wt[:, 0:512].rearrange("p (s q) -> p s q", s=2),
                                     axis=AX.X)
                nc.vector.reduce_sum(outb[:, base + 2:base + 4],
                                     wt[:, 512:768].rearrange("p (s q) -> p s q", s=2),
                                     axis=AX.X)
        nc.sync.dma_start(dd['out_d'][:], outb[:])


def _prep_in_maps(inputs):
    f = lambda k: np.ascontiguousarray(np.asarray(inputs[k], np.float32))
    bf = lambda a: np.ascontiguousarray(np.asarray(a, ml_dtypes.bfloat16))
    ctx_e, asp_e = f('context_inputs_embeds'), f('aspect_inputs_embeds')
    fc1_w, fc1_b = f('fc1_w'), f('fc1_b')
    pos = f('pos_emb')

    fc1p = np.zeros((384, D), np.float32)
    fc1p[:EMB] = fc1_w
    posT = np.concatenate([pos[:256].T, pos[:256].T, pos[:128].T, pos[:128].T],
                          axis=1) + fc1_b[:, None]
    W1 = f('W1')
    # [L, FFC, 128, 768]: W1r[l, fc, p, kc*128+c2] = W1[l, kc*128+p, fc*128+c2]
    W1r = W1.reshape(L, KC, 128, FFC, 128).transpose(0, 3, 2, 1, 4).reshape(L, FFC, 128, D)
    common = {
        'fc1p': bf(fc1p), 'posT': bf(posT),
        'emb_g': f('emb_g'), 'emb_b': f('emb_b'),
        'Wq': bf(f('Wq')), 'Wk': bf(f('Wk')), 'Wv': bf(f('Wv')), 'Wo': bf(f('Wo')),
        'bq': f('bq'), 'bk8': f('bk') * 0.125, 'bv': f('bv'), 'bo': f('bo'),
        'ln1_g': f('ln1_g'), 'ln1_b': f('ln1_b'),
        'ln2_g': f('ln2_g'), 'ln2_b': f('ln2_b'),
        'W1r': bf(W1r), 'W2': bf(f('W2')), 'bf1': f('bf1'), 'bf2': f('bf2'),
    }
    in_maps = []
    for i in range(8):
        xT = np.zeros((384, T), np.float32)
        xT[:EMB, 0:256] = ctx_e[2 * i].T
        xT[:EMB, 256:512] = ctx_e[2 * i + 1].T
        xT[:EMB, 512:640] = asp_e[2 * i].T
        xT[:EMB, 640:768] = asp_e[2 * i + 1].T
        in_maps.append({**common, 'xT': bf(xT)})
    return in_maps


def _postprocess(results):
    out = np.zeros(16, np.float32)
    for i, r in enumerate(results):
        A = r['out'].reshape(128, 24, 4)
        for j in range(2):
            out[2 * i + j] = float(np.sum(
                A[:, :, 2 + j].astype(np.float64) * A[:, :, j].astype(np.float64)))
    return out


def get_nc(n_layers=L, taps=(), with_bias=False):
    key = (n_layers, tuple(taps), with_bias)
    if key not in _NC_CACHE:
        _NC_CACHE[key] = _build_nc(n_layers, taps, with_bias)
    return _NC_CACHE[key]


def kernel(**inputs):
    wb = any(float(np.abs(np.asarray(inputs[k])).max()) > 0
             for k in ('bv', 'bo', 'bf2'))
    nc = get_nc(with_bias=wb)
    in_maps = _prep_in_maps(inputs)
    res = run_bass_kernel_spmd(nc, in_maps, CORES)
    return _postprocess(res.results)


if __name__ == "__main__":
    d = np.load('/root/problem/inputs_cache.npz')
    out = kernel(**{k: d[k] for k in d.files})
    ref = np.load('/root/problem/ref_out.npy')
    rel = np.abs(out - ref) / np.abs(ref)
    print("out:", out)
    print("rel err:", rel.max())


# revision 16
# speedup vs baseline: 1.2649x; 1.0040x over previous
"""Trainium2 Bass kernel for nn_AspectLinkModel (BERT-ish dual encoder + pairwise cosine sum).

Strategy: pure data-parallel over batch (2 batches/core x 8 cores), both encoder
calls fused into one 768-token stream per core. Feature-major activations,
bf16 matmul operands (converted on host) with f32 accumulation. Output [16]
assembled on host from per-core per-sequence normalized sums (sum-then-dot).
"""
import sys
import numpy as np
import ml_dtypes

for _p in ('/opt/trn_rl_repo', '/root/.axon_site/_ro/trn_rl_repo'):
    if _p not in sys.path:
        sys.path.insert(0, _p)

import concourse.bass as bass  # noqa: E402
import concourse.tile as tile  # noqa: E402
from concourse import bacc, mybir  # noqa: E402
from concourse.bass_utils import run_bass_kernel_spmd  # noqa: E402

# Steer the ACT table-set chooser so interleaved Exp/Ln activations resolve to
# the combined natural_log_exp_and_others set instead of thrashing between
# exp_and_others and natural_log (~2.7us per reload). Set order (and therefore
# act_func_set_id indices) is preserved; we only hide exp/ln from the
# single-function sets.
import concourse.hw_specs as _hw_specs  # noqa: E402
import functools  # noqa: E402

_orig_get_tables = _hw_specs.get_activation_tables


@functools.cache
def _patched_get_tables(arch):
    src = _orig_get_tables(arch)
    out = {}
    for name, funcs in src.items():
        fs = set(funcs)
        if name == 'exp_and_others':
            fs.discard(mybir.ActivationFunctionType.Exp)
        if name == 'natural_log':
            fs.discard(mybir.ActivationFunctionType.Ln)
        out[name] = fs
    return out


_hw_specs.get_activation_tables = _patched_get_tables
if getattr(bacc, 'get_activation_tables', None) is _orig_get_tables:
    bacc.get_activation_tables = _patched_get_tables

F32 = mybir.dt.float32
BF16 = mybir.dt.bfloat16
AF = mybir.ActivationFunctionType
AX = mybir.AxisListType

L, D, H, DH, FF, EMB = 4, 768, 12, 64, 3072, 300
KC = D // 128          # 6 feature chunks
FFC = FF // 128        # 24 ffn chunks
T = 768                # tokens per core: [ctx0(256) | ctx1(256) | asp0(128) | asp1(128)]
NTS = [(0, 384), (384, 384)]
CORES = list(range(8))

_NC_CACHE = {}


def _build_nc(n_layers=L, taps=(), with_bias=False):
    nc = bacc.Bacc("TRN2", target_bir_lowering=False, debug=False)

    dd = {}
    dd['xT_d'] = nc.dram_tensor("xT", [384, T], BF16, kind="ExternalInput")
    dd['fc1_d'] = nc.dram_tensor("fc1p", [384, D], BF16, kind="ExternalInput")
    dd['posT_d'] = nc.dram_tensor("posT", [D, T], BF16, kind="ExternalInput")
    dd['embg_d'] = nc.dram_tensor("emb_g", [D], F32, kind="ExternalInput")
    dd['embb_d'] = nc.dram_tensor("emb_b", [D], F32, kind="ExternalInput")
    dd['Wq_d'] = nc.dram_tensor("Wq", [L, D, D], BF16, kind="ExternalInput")
    dd['Wk_d'] = nc.dram_tensor("Wk", [L, D, D], BF16, kind="ExternalInput")
    dd['Wv_d'] = nc.dram_tensor("Wv", [L, D, D], BF16, kind="ExternalInput")
    dd['Wo_d'] = nc.dram_tensor("Wo", [L, D, D], BF16, kind="ExternalInput")
    dd['bq_d'] = nc.dram_tensor("bq", [L, D], F32, kind="ExternalInput")
    dd['bk_d'] = nc.dram_tensor("bk8", [L, D], F32, kind="ExternalInput")
    dd['bv_d'] = nc.dram_tensor("bv", [L, D], F32, kind="ExternalInput")
    dd['bo_d'] = nc.dram_tensor("bo", [L, D], F32, kind="ExternalInput")
    dd['l1g_d'] = nc.dram_tensor("ln1_g", [L, D], F32, kind="ExternalInput")
    dd['l1b_d'] = nc.dram_tensor("ln1_b", [L, D], F32, kind="ExternalInput")
    dd['l2g_d'] = nc.dram_tensor("ln2_g", [L, D], F32, kind="ExternalInput")
    dd['l2b_d'] = nc.dram_tensor("ln2_b", [L, D], F32, kind="ExternalInput")
    dd['W1r_d'] = nc.dram_tensor("W1r", [L, FFC, 128, D], BF16, kind="ExternalInput")
    dd['W2_d'] = nc.dram_tensor("W2", [L, FF, D], BF16, kind="ExternalInput")
    dd['bf1_d'] = nc.dram_tensor("bf1", [L, FF], F32, kind="ExternalInput")
    dd['bf2_d'] = nc.dram_tensor("bf2", [L, D], F32, kind="ExternalInput")
    dd['out_d'] = nc.dram_tensor("out", [128, 96], F32, kind="ExternalOutput")
    tap_d = {}
    for tname in taps:
        tap_d[tname] = nc.dram_tensor("tap_" + tname, [128, 6 * T], BF16,
                                      kind="ExternalOutput")

    with tile.TileContext(nc) as tc:
        _body(nc, tc, dd, n_layers, taps, tap_d, with_bias)
    nc.compile()
    return nc


def _body(nc, tc, dd, n_layers, taps, tap_d, with_bias):
    import contextlib
    ctx = contextlib.ExitStack()
    with ctx:
        sb = ctx.enter_context(tc.tile_pool(name="sb", bufs=1))
        ps = ctx.enter_context(tc.tile_pool(name="ps", bufs=1, space="PSUM"))

        # ---------------- constants ----------------
        ones_bf = sb.tile([128, 128], BF16, name="ones_bf", tag="const", bufs=4)
        nc.vector.memset(ones_bf[:], 1.0)
        eps12 = sb.tile([128, 1], F32, name="eps12", tag="const_e", bufs=2)
        nc.vector.memset(eps12[:], 1e-12)
        eps16 = sb.tile([128, 1], F32, name="eps16", tag="const_e2", bufs=2)
        nc.vector.memset(eps16[:], 1e-16)

        def col_tile(name, dram_row, n):
            t = sb.tile([128, n], F32, name=name, tag="cols", bufs=44)
            nc.sync.dma_start(t[:], dram_row.rearrange("(c p) -> p c", p=128))
            return t

        # 6 persistent big bf16 tiles [128, 6*768]; each layer uses 4 of them
        # (qT, kT, vtok, oT -> then f1 parts), rotated by 4 per layer so that
        # a layer's early writes never wait on the previous layer's F2 reads.
        bigs6 = [sb.tile([128, KC * T], BF16, name=f"bigp{i}", tag=f"pers{i}", bufs=1)
                 for i in range(6)]

        def layer_bigs(l):
            return [bigs6[(4 * l + i) % 6] for i in range(4)]

        def hpool(name):
            return sb.tile([128, KC * T], BF16, name=name, tag="hmid", bufs=2)

        def saved_tile(name):
            return sb.tile([128, KC * T], BF16, name=name, tag="saved", bufs=4)

        def s_tile(name):
            return sb.tile([128, KC * T], BF16, name=name, tag="spre", bufs=1)

        def wstr(name):
            return sb.tile([128, D], BF16, name=name, tag="wstr", bufs=9)

        def bank(name, w=384):
            return ps.tile([128, w], F32, name=name, tag="bank", bufs=8)

        def stat(name):
            return sb.tile([128, 384], F32, name=name, tag="stat", bufs=8)

        def statw(name):
            return sb.tile([128, D], F32, name=name, tag="statw", bufs=3)

        def stbf(name):
            return sb.tile([128, 384], BF16, name=name, tag="stbf", bufs=4)

        # ---------------- LayerNorm (token-half pipelined) ----------------
        def layer_norm(pref, s, g_col, gc, b_col, bc, out):
            """s: [128,4608] bf16 pre-LN sums; out: [128,4608] bf16.
            Processed independently per token half so consumers of half 0
            can start while half 1 is still in flight."""
            for ih, (o, w) in enumerate(NTS):
                S1 = bank(f"{pref}_S1_{ih}")
                S2 = bank(f"{pref}_S2_{ih}")
                for c in range(KC):
                    sq = stbf(f"{pref}_sq{ih}_{c}")
                    nc.gpsimd.tensor_mul(sq[:], s[:, c * T + o:c * T + o + w],
                                         s[:, c * T + o:c * T + o + w])
                    nc.tensor.matmul(S1[:], ones_bf[:], s[:, c * T + o:c * T + o + w],
                                     start=(c == 0), stop=(c == KC - 1))
                    nc.tensor.matmul(S2[:], ones_bf[:], sq[:],
                                     start=(c == 0), stop=(c == KC - 1))
                mean = stat(f"{pref}_mean{ih}")
                m2 = stat(f"{pref}_m2{ih}")
                nc.scalar.mul(mean[:], S1[:], 1.0 / D)
                nc.scalar.mul(m2[:], S2[:], 1.0 / D)
                var = stat(f"{pref}_var{ih}")
                nc.vector.tensor_mul(var[:], mean[:], mean[:])
                nc.vector.tensor_sub(var[:], m2[:], var[:])
                # rstd = exp(-0.5 * ln(var + eps))  (ACT, keeps DVE free)
                lnv = stat(f"{pref}_lnv{ih}")
                nc.scalar.activation(lnv[:], var[:], AF.Ln, bias=eps12[:])
                rstd = stat(f"{pref}_rstd{ih}")
                nc.scalar.activation(rstd[:], lnv[:], AF.Exp, scale=-0.5)
                for c in range(KC):
                    eng = nc.gpsimd if c >= 4 else nc.vector
                    t1 = stat(f"{pref}_t1_{ih}_{c}")
                    eng.tensor_sub(t1[:], s[:, c * T + o:c * T + o + w], mean[:])
                    eng.tensor_mul(t1[:], t1[:], rstd[:])
                    nc.scalar.activation(out[:, c * T + o:c * T + o + w], t1[:],
                                         AF.Identity, bias=b_col[:, bc:bc + 1],
                                         scale=g_col[:, gc:gc + 1])

        # ---------------- embed: fc1 + pos + LN ----------------
        xT_bf = [wstr(f"xT{c}") for c in range(3)]
        fc1_bf = [wstr(f"fc1w{c}") for c in range(3)]
        for c in range(3):
            nc.sync.dma_start(xT_bf[c][:], dd['xT_d'][c * 128:(c + 1) * 128, :])
            nc.sync.dma_start(fc1_bf[c][:], dd['fc1_d'][c * 128:(c + 1) * 128, :])
        posT = bigs6[0]
        for c in range(KC):
            nc.sync.dma_start(posT[:, c * T:(c + 1) * T],
                              dd['posT_d'][c * 128:(c + 1) * 128, :])
        embg = col_tile("embg", dd['embg_d'][:], KC)
        embb = col_tile("embb", dd['embb_d'][:], KC)

        s0 = s_tile("s_emb")
        for i, (o, w) in enumerate(NTS):
            for mc in range(KC):
                p = bank(f"emb_{mc}_{i}")
                for kc in range(3):
                    nc.tensor.matmul(p[:], fc1_bf[kc][:, mc * 128:(mc + 1) * 128],
                                     xT_bf[kc][:, o:o + w],
                                     start=(kc == 0), stop=(kc == 2))
                nc.vector.tensor_add(s0[:, mc * T + o:mc * T + o + w], p[:],
                                     posT[:, mc * T + o:mc * T + o + w])
        h = hpool("h0")
        layer_norm("ln_emb", s0, embg, 0, embb, 0, h)
        if 'h0' in taps:
            nc.sync.dma_start(tap_d['h0'][:], h[:])

        saved = []
        N2 = [bank(f"n2_{i}") for i in range(2)]

        # ---------------- transformer layers ----------------
        for l in range(n_layers):
            bq = col_tile(f"bq{l}", dd['bq_d'][l], KC)
            bk = col_tile(f"bk{l}", dd['bk_d'][l], KC)
            l1g = col_tile(f"l1g{l}", dd['l1g_d'][l], KC)
            l1b = col_tile(f"l1b{l}", dd['l1b_d'][l], KC)
            l2g = col_tile(f"l2g{l}", dd['l2g_d'][l], KC)
            l2b = col_tile(f"l2b{l}", dd['l2b_d'][l], KC)
            bf1c = col_tile(f"bf1{l}", dd['bf1_d'][l], FFC)
            if with_bias:
                bvc = col_tile(f"bvc{l}", dd['bv_d'][l], KC)
                boc = col_tile(f"boc{l}", dd['bo_d'][l], KC)
                bf2c = col_tile(f"bf2c{l}", dd['bf2_d'][l], KC)

            # ---- Q^T, K^T (feature-major, K pre-scaled 1/8) ----
            lb = layer_bigs(l)
            qT = lb[0]
            kT = lb[1]
            for dst, W_d, bias_col, scl, wn in ((qT, dd['Wq_d'], bq, 1.0, 'q'),
                                                (kT, dd['Wk_d'], bk, 0.125, 'k')):
                w_bf = [wstr(f"w{wn}{l}_{c}") for c in range(KC)]
                for c in range(KC):
                    nc.sync.dma_start(w_bf[c][:], W_d[l, c * 128:(c + 1) * 128, :])
                for i, (o, w) in enumerate(NTS):
                    for mc in range(KC):
                        p = bank(f"p{wn}{l}_{mc}_{i}")
                        for kc in range(KC):
                            nc.tensor.matmul(p[:], w_bf[kc][:, mc * 128:(mc + 1) * 128],
                                             h[:, kc * T + o:kc * T + o + w],
                                             start=(kc == 0), stop=(kc == KC - 1))
                        if i == 0:
                            nc.scalar.activation(dst[:, mc * T + o:mc * T + o + w], p[:],
                                                 AF.Identity, bias=bias_col[:, mc:mc + 1],
                                                 scale=scl)
                        else:
                            nc.vector.tensor_scalar(
                                dst[:, mc * T + o:mc * T + o + w], p[:], scl,
                                bias_col[:, mc:mc + 1], mybir.AluOpType.mult,
                                mybir.AluOpType.add)

            # ---- V (token-major: [token_chunk rows, feature cols]) ----
            vtok = lb[2]
            wv_bf = [wstr(f"wv{l}_{c}") for c in range(KC)]
            for c in range(KC):
                nc.sync.dma_start(wv_bf[c][:], dd['Wv_d'][l, c * 128:(c + 1) * 128, :])
            for tch in range(KC):
                for i, (o, w) in enumerate(NTS):
                    p = bank(f"pv{l}_{tch}_{i}")
                    for kc in range(KC):
                        nc.tensor.matmul(p[:], h[:, kc * T + tch * 128:kc * T + (tch + 1) * 128],
                                         wv_bf[kc][:, o:o + w],
                                         start=(kc == 0), stop=(kc == KC - 1))
                    nc.vector.tensor_copy(vtok[:, tch * T + o:tch * T + o + w], p[:])

            # ---- attention (heads paired per feature chunk) ----
            oT = lb[3]
            for si, (qo, kts) in enumerate(((0, (0, 1)), (256, (2, 3)))):
                for c in range(KC):
                    av = bank(f"av{l}_{si}_{c}", 256)
                    cs = bank(f"cs{l}_{si}_{c}", 256)
                    eTs = []
                    for ki, kt in enumerate(kts):
                        eT = sb.tile([128, 512], BF16, name=f"eT{l}_{si}_{c}_{ki}",
                                     tag="eT", bufs=6)
                        for j in range(2):
                            sp = bank(f"sp{l}_{si}_{c}_{ki}_{j}", 256)
                            nc.tensor.matmul(
                                sp[:],
                                kT[j * 64:j * 64 + 64, c * T + kt * 128:c * T + (kt + 1) * 128],
                                qT[j * 64:j * 64 + 64, c * T + qo:c * T + qo + 256],
                                start=True, stop=True)
                            nc.scalar.activation(eT[:, j * 256:(j + 1) * 256], sp[:], AF.Exp)
                        eTs.append(eT)
                    for j in range(2):
                        hh = 2 * c + j
                        r0 = j * 64
                        for ki in range(2):
                            nc.tensor.matmul(av[r0:r0 + 64, :],
                                             vtok[:, kts[ki] * T + hh * 64:kts[ki] * T + hh * 64 + 64],
                                             eTs[ki][:, j * 256:(j + 1) * 256],
                                             start=(ki == 0), stop=(ki == 1),
                                             tile_position=(0, r0))
                        for ki in range(2):
                            nc.tensor.matmul(cs[r0:r0 + 64, :], ones_bf[:, 0:64],
                                             eTs[ki][:, j * 256:(j + 1) * 256],
                                             start=(ki == 0), stop=(ki == 1),
                                             tile_position=(0, r0))
                    # inv = exp(-ln(cs)) on ACT (frees DVE); multiply on DVE (PSUM)
                    lnc = sb.tile([128, 256], F32, name=f"lnc{l}_{si}_{c}",
                                  tag="inv", bufs=4)
                    nc.scalar.activation(lnc[:], cs[:], AF.Ln)
                    inv = sb.tile([128, 256], F32, name=f"inv{l}_{si}_{c}",
                                  tag="inv", bufs=4)
                    nc.scalar.activation(inv[:], lnc[:], AF.Exp, scale=-1.0)
                    nc.vector.tensor_mul(oT[:, c * T + qo:c * T + qo + 256], av[:], inv[:])
            # asp joint pass: q = [512:768] covers both asp seqs; keys per chunk
            for kt in (4, 5):
                qw0 = 512 + (kt - 4) * 128
                col0 = (kt - 4) * 128
                for c in range(KC):
                    eT = sb.tile([128, 512], BF16, name=f"eTa{l}_{kt}_{c}",
                                 tag="eT", bufs=6)
                    for j in range(2):
                        sp = bank(f"spa{l}_{kt}_{c}_{j}", 256)
                        nc.tensor.matmul(
                            sp[:],
                            kT[j * 64:j * 64 + 64, c * T + kt * 128:c * T + (kt + 1) * 128],
                            qT[j * 64:j * 64 + 64, c * T + 512:c * T + 768],
                            start=True, stop=True)
                        nc.scalar.activation(eT[:, j * 256:(j + 1) * 256], sp[:], AF.Exp)
                    av = bank(f"ava{l}_{kt}_{c}", 128)
                    cs = bank(f"csa{l}_{kt}_{c}", 128)
                    for j in range(2):
                        hh = 2 * c + j
                        r0 = j * 64
                        nc.tensor.matmul(av[r0:r0 + 64, :],
                                         vtok[:, kt * T + hh * 64:kt * T + hh * 64 + 64],
                                         eT[:, j * 256 + col0:j * 256 + col0 + 128],
                                         start=True, stop=True, tile_position=(0, r0))
                        nc.tensor.matmul(cs[r0:r0 + 64, :], ones_bf[:, 0:64],
                                         eT[:, j * 256 + col0:j * 256 + col0 + 128],
                                         start=True, stop=True, tile_position=(0, r0))
                    lnc = sb.tile([128, 256], F32, name=f"lnca{l}_{kt}_{c}",
                                  tag="inv", bufs=4)
                    nc.scalar.activation(lnc[:, 0:128], cs[:], AF.Ln)
                    inv = sb.tile([128, 256], F32, name=f"inva{l}_{kt}_{c}",
                                  tag="inv", bufs=4)
                    nc.scalar.activation(inv[:, 0:128], lnc[:, 0:128], AF.Exp, scale=-1.0)
                    nc.vector.tensor_mul(oT[:, c * T + qw0:c * T + qw0 + 128],
                                         av[:], inv[:, 0:128])

            if with_bias:
                for c in range(KC):
                    nc.scalar.activation(oT[:, c * T:(c + 1) * T],
                                         oT[:, c * T:(c + 1) * T], AF.Identity,
                                         bias=bvc[:, c:c + 1])

            # ---- O projection + residual -> s1, LN1 -> hmid ----
            wo_bf = [wstr(f"wo{l}_{c}") for c in range(KC)]
            for c in range(KC):
                nc.sync.dma_start(wo_bf[c][:], dd['Wo_d'][l, c * 128:(c + 1) * 128, :])
            s1 = s_tile(f"s1_{l}")
            for i, (o, w) in enumerate(NTS):
                for mc in range(KC):
                    p = bank(f"po{l}_{mc}_{i}")
                    for kc in range(KC):
                        nc.tensor.matmul(p[:], wo_bf[kc][:, mc * 128:(mc + 1) * 128],
                                         oT[:, kc * T + o:kc * T + o + w],
                                         start=(kc == 0), stop=(kc == KC - 1))
                    nc.vector.tensor_add(s1[:, mc * T + o:mc * T + o + w], p[:],
                                         h[:, mc * T + o:mc * T + o + w])
                    if with_bias:
                        nc.scalar.activation(s1[:, mc * T + o:mc * T + o + w],
                                             s1[:, mc * T + o:mc * T + o + w],
                                             AF.Identity, bias=boc[:, mc:mc + 1])
            hmid = hpool(f"hmid{l}")
            layer_norm(f"ln1_{l}", s1, l1g, 0, l1b, 0, hmid)

            # ---- FFN ----
            w2all = sb.tile([128, FFC * T], BF16, name=f"w2all{l}", tag="w2all", bufs=1)
            for fc in range(FFC):
                nc.sync.dma_start(w2all[:, fc * T:(fc + 1) * T],
                                  dd['W2_d'][l, fc * 128:(fc + 1) * 128, :])
            f1 = lb
            for fc in range(FFC):
                w1c = wstr(f"w1_{l}_{fc}")
                nc.sync.dma_start(w1c[:], dd['W1r_d'][l, fc])
                ftile, foff = f1[fc // 6], (fc % 6) * T
                for i, (o, w) in enumerate(NTS):
                    p = bank(f"pf1{l}_{fc}_{i}")
                    for kc in range(KC):
                        nc.tensor.matmul(p[:], w1c[:, kc * 128:(kc + 1) * 128],
                                         hmid[:, kc * T + o:kc * T + o + w],
                                         start=(kc == 0), stop=(kc == KC - 1))
                    nc.scalar.activation(ftile[:, foff + o:foff + o + w], p[:],
                                         AF.Gelu_apprx_tanh, bias=bf1c[:, fc:fc + 1])
            s2 = s_tile(f"s2_{l}")
            for i, (o, w) in enumerate(NTS):
                for mc in range(KC):
                    p = bank(f"pf2{l}_{mc}_{i}")
                    for fc in range(FFC):
                        ftile, foff = f1[fc // 6], (fc % 6) * T
                        nc.tensor.matmul(
                            p[:], w2all[:, fc * T + mc * 128:fc * T + (mc + 1) * 128],
                            ftile[:, foff + o:foff + o + w],
                            start=(fc == 0), stop=(fc == FFC - 1))
                    nc.vector.tensor_add(s2[:, mc * T + o:mc * T + o + w], p[:],
                                         hmid[:, mc * T + o:mc * T + o + w])
                    if with_bias:
                        nc.scalar.activation(s2[:, mc * T + o:mc * T + o + w],
                                             s2[:, mc * T + o:mc * T + o + w],
                                             AF.Identity, bias=bf2c[:, mc:mc + 1])
            hout = saved_tile(f"hL{l}")
            layer_norm(f"ln2_{l}", s2, l2g, 0, l2b, 0, hout)
            # accumulate squared-norm contribution for the final cosine stage
            for c in range(KC):
                for i, (o, w) in enumerate(NTS):
                    sq = stbf(f"fin_sq{l}_{c}_{i}")
                    nc.gpsimd.tensor_mul(sq[:], hout[:, c * T + o:c * T + o + w],
                                         hout[:, c * T + o:c * T + o + w])
                    nc.tensor.matmul(N2[i][:], ones_bf[:], sq[:],
                                     start=(l == 0 and c == 0),
                                     stop=(l == n_layers - 1 and c == KC - 1))
            saved.append(hout)
            h = hout
            if f"h{l + 1}" in taps:
                nc.sync.dma_start(tap_d[f"h{l + 1}"][:], hout[:])

        # ---------------- final: weighted per-seq sums ----------------
        invn = statw("invn")
        for i, (o, w) in enumerate(NTS):
            sd = stat(f"fin_sd{i}")
            nc.scalar.activation(sd[:], N2[i][:], AF.Sqrt, bias=eps16[:])
            nc.vector.reciprocal(invn[:, o:o + w], sd[:])
        outb = sb.tile([128, 96], F32, name="outb", tag="outb", bufs=1)
        nc.vector.memset(outb[:], 0.0)
        for li, hs in enumerate(saved):
            for c in range(KC):
                wt = statw(f"fin_w{li}_{c}")
                eng = nc.vector if (c % 2 == 0) else nc.gpsimd
                eng.tensor_mul(wt[:], hs[:, c * T:(c + 1) * T], invn[:])
                base = (li * KC + c) * 4
                nc.vector.reduce_sum(outb[:, base:base + 2],
                                     wt[:, 0:512].rearrange("p (s q) -> p s q", s=2),
                                     axis=AX.X)
                nc.vector.reduce_sum(outb[:, base + 2:base + 4],
                                     wt[:, 512:768].rearrange("p (s q) -> p s q", s=2),
                                     axis=AX.X)
        nc.sync.dma_start(dd['out_d'][:], outb[:])


def _prep_in_maps(inputs):
    f = lambda k: np.ascontiguousarray(np.asarray(inputs[k], np.float32))
    bf = lambda a: np.ascontiguousarray(np.asarray(a, ml_dtypes.bfloat16))
    ctx_e, asp_e = f('context_inputs_embeds'), f('aspect_inputs_embeds')
    fc1_w, fc1_b = f('fc1_w'), f('fc1_b')
    pos = f('pos_emb')

    fc1p = np.zeros((384, D), np.float32)
    fc1p[:EMB] = fc1_w
    posT = np.concatenate([pos[:256].T, pos[:256].T, pos[:128].T, pos[:128].T],
                          axis=1) + fc1_b[:, None]
    W1 = f('W1')
    # [L, FFC, 128, 768]: W1r[l, fc, p, kc*128+c2] = W1[l, kc*128+p, fc*128+c2]
    W1r = W1.reshape(L, KC, 128, FFC, 128).transpose(0, 3, 2, 1, 4).reshape(L, FFC, 128, D)
    common = {
        'fc1p': bf(fc1p), 'posT': bf(posT),
        'emb_g': f('emb_g'), 'emb_b': f('emb_b'),
        'Wq': bf(f('Wq')), 'Wk': bf(f('Wk')), 'Wv': bf(f('Wv')), 'Wo': bf(f('Wo')),
        'bq': f('bq'), 'bk8': f('bk') * 0.125, 'bv': f('bv'), 'bo': f('bo'),
        'ln1_g': f('ln1_g'), 'ln1_b': f('ln1_b'),
        'ln2_g': f('ln2_g'), 'ln2_b': f('ln2_b'),
        'W1r': bf(W1r), 'W2': bf(f('W2')), 'bf1': f('bf1'), 'bf2': f('bf2'),
    }
    in_maps = []
    for i in range(8):
        xT = np.zeros((384, T), np.float32)
        xT[:EMB, 0:256] = ctx_e[2 * i].T
        xT[:EMB, 256:512] = ctx_e[2 * i + 1].T
        xT[:EMB, 512:640] = asp_e[2 * i].T
        xT[:EMB, 640:768] = asp_e[2 * i + 1].T
        in_maps.append({**common, 'xT': bf(xT)})
    return in_maps


def _postprocess(results):
    out = np.zeros(16, np.float32)
    for i, r in enumerate(results):
        A = r['out'].reshape(128, 24, 4)
        for j in range(2):
            out[2 * i + j] = float(np.sum(
                A[:, :, 2 + j].astype(np.float64) * A[:, :, j].astype(np.float64)))
    return out


def get_nc(n_layers=L, taps=(), with_bias=False):
    key = (n_layers, tuple(taps), with_bias)
    if key not in _NC_CACHE:
        _NC_CACHE[key] = _build_nc(n_layers, taps, with_bias)
    return _NC_CACHE[key]


def kernel(**inputs):
    wb = any(float(np.abs(np.asarray(inputs[k])).max()) > 0
             for k in ('bv', 'bo', 'bf2'))
    nc = get_nc(with_bias=wb)
    in_maps = _prep_in_maps(inputs)
    res = run_bass_kernel_spmd(nc, in_maps, CORES)
    return _postprocess(res.results)


if __name__ == "__main__":
    d = np.load('/root/problem/inputs_cache.npz')
    out = kernel(**{k: d[k] for k in d.files})
    ref = np.load('/root/problem/ref_out.npy')
    rel = np.abs(out - ref) / np.abs(ref)
    print("out:", out)
    print("rel err:", rel.max())
